# revision 7
# baseline (speedup 1.0000x reference)
"""Trainium2 Bass kernel for nn_AttentionEncoder (8-core SPMD).

Structure:
  Phase A (tensor-parallel over conv1 output channels):
    h[b, o] = sum_k x[b, k] * W1[o, k]  -- the 3.28 GB W1 stream dominates
    (memory-bound).  W1 is sharded into 8 x [338, 303750] output-channel
    slices; each core streams its slice (cast to fp16 on host, halving
    HBM traffic) through the PE as the moving operand while the (tiny)
    transposed x is the stationary operand.  BatchNorm (+conv1 bias) is
    folded into a per-channel scale/bias applied in the epilogue.
  Phase B (data-parallel over batch, 2 per core):
    logits = h @ W2.T + b2 ; gumbel-softmax over N=9 ; attention pooling
    dot[b,u,f] = sum_n prob[b,u,n] * input[b,u,n,f] done as a block-
    diagonal [81x9] matmul against input[b] viewed as [81, 3750].
"""

import os
import sys
import tempfile

import numpy as np

for _p in ("/opt/trn_rl_repo", "/root/.axon_site/_ro/trn_rl_repo"):
    if os.path.isdir(_p) and _p not in sys.path:
        sys.path.append(_p)

import concourse.tile as tile
from concourse import bacc, mybir
from concourse.bass_utils import run_bass_kernel_spmd

# ---- problem constants (hardcoded; kernel.py must be self-contained) ----
B, U, A, N, F, L = 16, 9, 1, 9, 3750, 300
K1 = U * N * F            # 303750  conv1 contraction
O1 = U * L                # 2700    conv1 output channels
O2 = U * A * N            # 81      conv2 output channels
BN_EPS = 1e-5
NCORES = 8
OS = 338                  # per-core conv1 output-channel shard (8*338=2704)
KT = 128                  # PE contraction tile
NKT = (K1 + KT - 1) // KT  # 2374 k-tiles (last one padded with zeros)
K1P = NKT * KT            # 303872
G = 16                    # k-tiles per W1 DMA chunk (1.385 MB per chunk)
BS = B // NCORES          # 2 batches per core in phase B
NKT2 = (O1 + KT - 1) // KT  # 22
K2P = NKT2 * KT           # 2816
FCH = 512                 # pooling free-dim chunk (one PSUM bank of fp32)

PROFILE = os.environ.get("BASS_KERNEL_PROFILE", "0") == "1"
LAST_EXEC_NS = {}

_cache = {}


def _register_profile_hook():
    """boot() skips NTFF hook registration when antenv.axon_hooks is absent;
    recreate the module and register the ctypes-based hook ourselves."""
    import types

    if "antenv.axon_hooks" in sys.modules:
        return
    mod = types.ModuleType("antenv.axon_hooks")
    _hook = [None]
    mod.set_axon_ntff_profile_hook = lambda h: _hook.__setitem__(0, h)
    mod.get_axon_ntff_profile_hook = lambda: _hook[0]
    sys.modules["antenv.axon_hooks"] = mod
    import antenv

    antenv.axon_hooks = mod
    try:
        from trn_agent_boot.trn_boot import _ntff_profile_via_ctypes

        mod.set_axon_ntff_profile_hook(
            _ntff_profile_via_ctypes("/opt/axon/libaxon_pjrt.so")
        )
    except Exception:
        pass
    import concourse.bass_utils as bu

    bu.upload_artifacts = lambda tmpdir: "local://" + tmpdir


def _build_phase_a():
    nc = bacc.Bacc("TRN2", target_bir_lowering=False, debug=False,
                   num_devices=NCORES)
    f16, f32 = mybir.dt.float16, mybir.dt.float32
    w1t = nc.dram_tensor("w1t", [NKT, KT, OS], f16, kind="ExternalInput").ap()
    xsb = nc.dram_tensor("xsb", [KT, NKT * B], f16, kind="ExternalInput").ap()
    ssb = nc.dram_tensor("ssb", [B, OS], f32, kind="ExternalInput").ap()
    tsb = nc.dram_tensor("tsb", [B, OS], f32, kind="ExternalInput").ap()
    hout = nc.dram_tensor("h", [B, OS], f32, kind="ExternalOutput").ap()

    with tile.TileContext(nc) as tc:
        with tc.tile_pool(name="xp", bufs=1) as xp, \
             tc.tile_pool(name="wp", bufs=6) as wp, \
             tc.tile_pool(name="pp", bufs=1, space="PSUM") as pp, \
             tc.tile_pool(name="ep", bufs=1) as ep:
            xt = xp.tile([KT, NKT * B], f16)
            nc.sync.dma_start(out=xt[:], in_=xsb)
            psum = pp.tile([B, OS], f32)
            for c in range((NKT + G - 1) // G):
                g0 = c * G
                gg = min(G, NKT - g0)
                wt = wp.tile([KT, G, OS], f16, tag="wt")
                nc.sync.dma_start(
                    out=wt[:, :gg, :],
                    in_=w1t[g0:g0 + gg].rearrange("g p j -> p g j"),
                )
                for g in range(gg):
                    t = g0 + g
                    nc.tensor.matmul(
                        psum[:],
                        lhsT=xt[:, t * B:(t + 1) * B],
                        rhs=wt[:, g, :],
                        start=(t == 0),
                        stop=(t == NKT - 1),
                    )
            st = ep.tile([B, OS], f32, tag="st")
            nc.sync.dma_start(out=st[:], in_=ssb)
            tt = ep.tile([B, OS], f32, tag="tt")
            nc.sync.dma_start(out=tt[:], in_=tsb)
            ho = ep.tile([B, OS], f32, tag="ho")
            nc.vector.tensor_mul(out=ho[:], in0=psum[:], in1=st[:])
            nc.vector.tensor_add(out=ho[:], in0=ho[:], in1=tt[:])
            nc.sync.dma_start(out=hout, in_=ho[:])
    nc.compile()
    return nc


def _build_phase_b():
    nc = bacc.Bacc("TRN2", target_bir_lowering=False, debug=False,
                   num_devices=NCORES)
    f32 = mybir.dt.float32
    hsb = nc.dram_tensor("hsb", [KT, NKT2 * BS], f32, kind="ExternalInput").ap()
    w2sb = nc.dram_tensor("w2sb", [KT, NKT2 * O2], f32, kind="ExternalInput").ap()
    addv = nc.dram_tensor("addv", [BS, O2], f32, kind="ExternalInput").ap()
    itemp = nc.dram_tensor("itemp", [BS, 1], f32, kind="ExternalInput").ap()
    minp = nc.dram_tensor("minp", [BS, O2, F], f32, kind="ExternalInput").ap()
    mask = nc.dram_tensor("mask", [O2, U], f32, kind="ExternalInput").ap()
    ident = nc.dram_tensor("ident", [BS, BS], f32, kind="ExternalInput").ap()
    dout = nc.dram_tensor("dot", [BS, U, F], f32, kind="ExternalOutput").ap()



    with tile.TileContext(nc) as tc:
        with tc.tile_pool(name="sb", bufs=1) as sb, \
             tc.tile_pool(name="inb", bufs=2) as ib, \
             tc.tile_pool(name="pp", bufs=1, space="PSUM") as pp:
            hs = sb.tile([KT, NKT2 * BS], f32, tag="hs")
            nc.sync.dma_start(out=hs[:], in_=hsb)
            w2 = sb.tile([KT, NKT2 * O2], f32, tag="w2")
            nc.sync.dma_start(out=w2[:], in_=w2sb)
            ps2 = pp.tile([BS, O2], f32, tag="ps2")
            for t in range(NKT2):
                nc.tensor.matmul(
                    ps2[:],
                    lhsT=hs[:, t * BS:(t + 1) * BS],
                    rhs=w2[:, t * O2:(t + 1) * O2],
                    start=(t == 0),
                    stop=(t == NKT2 - 1),
                )
            av = sb.tile([BS, O2], f32, tag="av")
            nc.sync.dma_start(out=av[:], in_=addv)
            it = sb.tile([BS, 1], f32, tag="it")
            nc.sync.dma_start(out=it[:], in_=itemp)
            ut = sb.tile([BS, O2], f32, tag="ut")
            # u = (logits * (1/temp)) + (b2 + gumbel)/temp
            nc.vector.scalar_tensor_tensor(
                out=ut[:], in0=ps2[:], scalar=it[:], in1=av[:],
                op0=mybir.AluOpType.mult, op1=mybir.AluOpType.add,
            )
            ea = sb.tile([BS, O2], f32, tag="ea")
            nc.scalar.activation(out=ea[:], in_=ut[:],
                                 func=mybir.ActivationFunctionType.Exp)
            den = sb.tile([BS, U], f32, tag="den")
            nc.vector.tensor_reduce(
                out=den[:],
                in_=ea[:].rearrange("p (u n) -> p u n", n=N),
                axis=mybir.AxisListType.X,
                op=mybir.AluOpType.add,
            )
            rec = sb.tile([BS, U], f32, tag="rec")
            nc.vector.reciprocal(out=rec[:], in_=den[:])
            idt = sb.tile([BS, BS], f32, tag="idt")
            nc.sync.dma_start(out=idt[:], in_=ident)
            psE = pp.tile([O2, BS], f32, tag="psE")
            nc.tensor.transpose(psE[:], ea[:], idt[:])
            etE = sb.tile([O2, BS], f32, tag="etE")
            nc.vector.tensor_copy(out=etE[:], in_=psE[:])
            psR = pp.tile([U, BS], f32, tag="psR")
            nc.tensor.transpose(psR[:], rec[:], idt[:])
            etR = sb.tile([U, BS], f32, tag="etR")
            nc.vector.tensor_copy(out=etR[:], in_=psR[:])
            mk = sb.tile([O2, U], f32, tag="mk")
            nc.sync.dma_start(out=mk[:], in_=mask)
            for b in range(BS):
                inb = ib.tile([O2, F], f32, tag="inb")
                nc.sync.dma_start(out=inb[:], in_=minp[b])
                pb = sb.tile([O2, U], f32, tag=f"pb{b}")
                # block-diagonal unnormalized prob: mask[un,u] * exp[un]
                nc.vector.tensor_scalar_mul(pb[:], mk[:], etE[:, b:b + 1])
                ob = sb.tile([U, F], f32, tag=f"ob{b}")
                for f0 in range(0, F, FCH):
                    w = min(FCH, F - f0)
                    psf = pp.tile([U, FCH], f32, tag="psf", bufs=4)
                    nc.tensor.matmul(psf[:, :w], lhsT=pb[:],
                                     rhs=inb[:, f0:f0 + w],
                                     start=True, stop=True)
                    # normalize by 1/den[u] during PSUM->SBUF copy
                    nc.vector.tensor_scalar_mul(
                        ob[:, f0:f0 + w], psf[:, :w], etR[:, b:b + 1])
                nc.sync.dma_start(out=dout[b], in_=ob[:])
    nc.compile()
    return nc


def _get_compiled():
    if "a" not in _cache:
        _cache["a"] = _build_phase_a()
    if "b" not in _cache:
        _cache["b"] = _build_phase_b()
    return _cache["a"], _cache["b"]


def _run(nc, in_maps, label):
    kw = {}
    if PROFILE:
        _register_profile_hook()
        kw = dict(trace=True, tmpdir=tempfile.mkdtemp(prefix=f"bass_{label}_"))
    res = run_bass_kernel_spmd(nc, in_maps, core_ids=list(range(NCORES)), **kw)
    if PROFILE:
        LAST_EXEC_NS[label] = res.exec_time_ns
    return res.results


def kernel(input, temp, W1, b1, gamma, beta, rmean, rvar, W2, b2, gumbel):
    input = np.ascontiguousarray(np.asarray(input, dtype=np.float32))
    temp = np.float32(np.asarray(temp))
    W1 = np.asarray(W1, dtype=np.float32)
    b1 = np.asarray(b1, dtype=np.float32)
    gamma = np.asarray(gamma, dtype=np.float32)
    beta = np.asarray(beta, dtype=np.float32)
    rmean = np.asarray(rmean, dtype=np.float32)
    rvar = np.asarray(rvar, dtype=np.float32)
    W2 = np.asarray(W2, dtype=np.float32)
    b2 = np.asarray(b2, dtype=np.float32)
    gumbel = np.asarray(gumbel, dtype=np.float32)

    nca, ncb = _get_compiled()

    # ---- host prep, phase A ----
    x2 = input.reshape(B, K1)
    xTp = np.zeros((K1P, B), np.float16)
    xTp[:K1] = x2.T
    xsb = np.ascontiguousarray(
        xTp.reshape(NKT, KT, B).transpose(1, 0, 2)).reshape(KT, NKT * B)

    s = (gamma.astype(np.float64) / np.sqrt(rvar.astype(np.float64) + BN_EPS))
    tv = s * (b1.astype(np.float64) - rmean.astype(np.float64)) \
        + beta.astype(np.float64)
    s = s.astype(np.float32)
    tv = tv.astype(np.float32)

    W1_2d = W1.reshape(O1, K1)
    in_maps_a = []
    for i in range(NCORES):
        o0 = i * OS
        o1 = min(o0 + OS, O1)
        ow = o1 - o0
        w1t_i = np.zeros((NKT, KT, OS), np.float16)
        w1t_i.reshape(K1P, OS)[:K1, :ow] = W1_2d[o0:o1].T
        sp = np.zeros((OS,), np.float32)
        sp[:ow] = s[o0:o1]
        tp = np.zeros((OS,), np.float32)
        tp[:ow] = tv[o0:o1]
        in_maps_a.append({
            "w1t": w1t_i,
            "xsb": xsb,
            "ssb": np.ascontiguousarray(np.repeat(sp[None], B, 0)),
            "tsb": np.ascontiguousarray(np.repeat(tp[None], B, 0)),
        })

    res_a = _run(nca, in_maps_a, "phase_a")
    h_full = np.concatenate([r["h"] for r in res_a], axis=1)[:, :O1]

    # ---- host prep, phase B ----
    hT = np.zeros((K2P, B), np.float32)
    hT[:O1] = h_full.T
    hT_r = np.ascontiguousarray(hT.reshape(NKT2, KT, B).transpose(1, 0, 2))
    W2_2d = W2.reshape(O2, O1)
    w2T = np.zeros((K2P, O2), np.float32)
    w2T[:O1] = W2_2d.T
    w2sb = np.ascontiguousarray(
        w2T.reshape(NKT2, KT, O2).transpose(1, 0, 2)).reshape(KT, NKT2 * O2)
    inv_t = np.float32(1.0) / temp
    gum2 = gumbel.reshape(B, O2)
    addv_all = (b2[None, :] + gum2) * inv_t
    maskm = np.zeros((O2, U), np.float32)
    maskm[np.arange(O2), np.arange(O2) // N] = 1.0
    identm = np.eye(BS, dtype=np.float32)
    itempm = np.full((BS, 1), inv_t, np.float32)
    inp81 = input.reshape(B, O2, F)

    in_maps_b = []
    for i in range(NCORES):
        b0 = i * BS
        hsb_i = np.ascontiguousarray(
            hT_r[:, :, b0:b0 + BS]).reshape(KT, NKT2 * BS)
        in_maps_b.append({
            "hsb": hsb_i,
            "w2sb": w2sb,
            "addv": np.ascontiguousarray(addv_all[b0:b0 + BS]),
            "itemp": itempm,
            "minp": np.ascontiguousarray(inp81[b0:b0 + BS]),
            "mask": maskm,
            "ident": identm,
        })

    res_b = _run(ncb, in_maps_b, "phase_b")
    out = np.concatenate([r["dot"] for r in res_b], axis=0)
    return out.reshape(B, U, A, F)


# revision 11
# speedup vs baseline: 1.2113x; 1.2113x over previous
"""Trainium2 Bass kernel for nn_AttentionEncoder (8-core SPMD).

Structure:
  Phase A (tensor-parallel over conv1 output channels):
    h[b, o] = sum_k x[b, k] * W1[o, k]  -- the 3.28 GB W1 stream dominates
    (memory-bound).  W1 is sharded into 8 x [338, 303750] output-channel
    slices; each core streams its slice (cast to fp16 on host, halving
    HBM traffic) through the PE as the moving operand while the (tiny)
    transposed x is the stationary operand.  BatchNorm (+conv1 bias) is
    folded into a per-channel scale/bias applied in the epilogue.
  Phase B (data-parallel over batch, 2 per core):
    logits = h @ W2.T + b2 ; gumbel-softmax over N=9 ; attention pooling
    dot[b,u,f] = sum_n prob[b,u,n] * input[b,u,n,f] done as a block-
    diagonal [81x9] matmul against input[b] viewed as [81, 3750].
"""

import os
import sys
import tempfile

import numpy as np

for _p in ("/opt/trn_rl_repo", "/root/.axon_site/_ro/trn_rl_repo"):
    if os.path.isdir(_p) and _p not in sys.path:
        sys.path.append(_p)

import concourse.tile as tile
from concourse import bacc, mybir
from concourse.bass_utils import run_bass_kernel_spmd

# ---- problem constants (hardcoded; kernel.py must be self-contained) ----
B, U, A, N, F, L = 16, 9, 1, 9, 3750, 300
K1 = U * N * F            # 303750  conv1 contraction
O1 = U * L                # 2700    conv1 output channels
O2 = U * A * N            # 81      conv2 output channels
BN_EPS = 1e-5
NCORES = 8
OS = 338                  # per-core conv1 output-channel shard (8*338=2704)
KT = 128                  # PE contraction tile
G = 16                    # k-tiles per W1 DMA chunk (1.385 MB per chunk)
NCH = (K1 + G * KT - 1) // (G * KT)  # 149 W1 chunks
NKT = NCH * G             # 2384 k-tiles (tail ones all-zero padding)
K1P = NKT * KT            # 305152
BS = B // NCORES          # 2 batches per core in phase B
NKT2 = (O1 + KT - 1) // KT  # 22
K2P = NKT2 * KT           # 2816
FCH = 512                 # pooling free-dim chunk (one PSUM bank of fp32)

PROFILE = os.environ.get("BASS_KERNEL_PROFILE", "0") == "1"
LAST_EXEC_NS = {}

_cache = {}


def _register_profile_hook():
    """boot() skips NTFF hook registration when antenv.axon_hooks is absent;
    recreate the module and register the ctypes-based hook ourselves."""
    import types

    if "antenv.axon_hooks" in sys.modules:
        return
    mod = types.ModuleType("antenv.axon_hooks")
    _hook = [None]
    mod.set_axon_ntff_profile_hook = lambda h: _hook.__setitem__(0, h)
    mod.get_axon_ntff_profile_hook = lambda: _hook[0]
    sys.modules["antenv.axon_hooks"] = mod
    import antenv

    antenv.axon_hooks = mod
    try:
        from trn_agent_boot.trn_boot import _ntff_profile_via_ctypes

        mod.set_axon_ntff_profile_hook(
            _ntff_profile_via_ctypes("/opt/axon/libaxon_pjrt.so")
        )
    except Exception:
        pass
    import concourse.bass_utils as bu

    bu.upload_artifacts = lambda tmpdir: "local://" + tmpdir


def _build_phase_a():
    nc = bacc.Bacc("TRN2", target_bir_lowering=False, debug=False,
                   num_devices=NCORES)
    f16, f32 = mybir.dt.float16, mybir.dt.float32
    w1t = nc.dram_tensor("w1t", [NCH, KT, G * OS], f16, kind="ExternalInput").ap()
    xsb = nc.dram_tensor("xsb", [KT, NKT * B], f16, kind="ExternalInput").ap()
    ssb = nc.dram_tensor("ssb", [B, OS], f32, kind="ExternalInput").ap()
    tsb = nc.dram_tensor("tsb", [B, OS], f32, kind="ExternalInput").ap()
    hout = nc.dram_tensor("h", [B, OS], f32, kind="ExternalOutput").ap()

    with tile.TileContext(nc) as tc:
        with tc.tile_pool(name="xp", bufs=1) as xp, \
             tc.tile_pool(name="wp", bufs=6) as wp, \
             tc.tile_pool(name="pp", bufs=1, space="PSUM") as pp, \
             tc.tile_pool(name="ep", bufs=1) as ep:
            xt = xp.tile([KT, NKT * B], f16)
            nc.sync.dma_start(out=xt[:], in_=xsb)
            psum = pp.tile([B, OS], f32)
            for c in range(NCH):
                wt = wp.tile([KT, G * OS], f16, tag="wt")
                nc.sync.dma_start(out=wt[:], in_=w1t[c])
                for g in range(G):
                    t = c * G + g
                    nc.tensor.matmul(
                        psum[:],
                        lhsT=xt[:, t * B:(t + 1) * B],
                        rhs=wt[:, g * OS:(g + 1) * OS],
                        start=(t == 0),
                        stop=(t == NKT - 1),
                    )
            st = ep.tile([B, OS], f32, tag="st")
            nc.sync.dma_start(out=st[:], in_=ssb)
            tt = ep.tile([B, OS], f32, tag="tt")
            nc.sync.dma_start(out=tt[:], in_=tsb)
            ho = ep.tile([B, OS], f32, tag="ho")
            nc.vector.tensor_mul(out=ho[:], in0=psum[:], in1=st[:])
            nc.vector.tensor_add(out=ho[:], in0=ho[:], in1=tt[:])
            nc.sync.dma_start(out=hout, in_=ho[:])
    nc.compile()
    return nc


def _build_phase_b():
    nc = bacc.Bacc("TRN2", target_bir_lowering=False, debug=False,
                   num_devices=NCORES)
    f32 = mybir.dt.float32
    hsb = nc.dram_tensor("hsb", [KT, NKT2 * BS], f32, kind="ExternalInput").ap()
    w2sb = nc.dram_tensor("w2sb", [KT, NKT2 * O2], f32, kind="ExternalInput").ap()
    addv = nc.dram_tensor("addv", [BS, O2], f32, kind="ExternalInput").ap()
    itemp = nc.dram_tensor("itemp", [BS, 1], f32, kind="ExternalInput").ap()
    minp = nc.dram_tensor("minp", [BS, O2, F], f32, kind="ExternalInput").ap()
    mask = nc.dram_tensor("mask", [O2, U], f32, kind="ExternalInput").ap()
    ident = nc.dram_tensor("ident", [BS, BS], f32, kind="ExternalInput").ap()
    dout = nc.dram_tensor("dot", [BS, U, F], f32, kind="ExternalOutput").ap()



    with tile.TileContext(nc) as tc:
        with tc.tile_pool(name="sb", bufs=1) as sb, \
             tc.tile_pool(name="inb", bufs=2) as ib, \
             tc.tile_pool(name="pp", bufs=1, space="PSUM") as pp:
            hs = sb.tile([KT, NKT2 * BS], f32, tag="hs")
            nc.sync.dma_start(out=hs[:], in_=hsb)
            w2 = sb.tile([KT, NKT2 * O2], f32, tag="w2")
            nc.sync.dma_start(out=w2[:], in_=w2sb)
            ps2 = pp.tile([BS, O2], f32, tag="ps2")
            for t in range(NKT2):
                nc.tensor.matmul(
                    ps2[:],
                    lhsT=hs[:, t * BS:(t + 1) * BS],
                    rhs=w2[:, t * O2:(t + 1) * O2],
                    start=(t == 0),
                    stop=(t == NKT2 - 1),
                )
            av = sb.tile([BS, O2], f32, tag="av")
            nc.sync.dma_start(out=av[:], in_=addv)
            it = sb.tile([BS, 1], f32, tag="it")
            nc.sync.dma_start(out=it[:], in_=itemp)
            ut = sb.tile([BS, O2], f32, tag="ut")
            # u = (logits * (1/temp)) + (b2 + gumbel)/temp
            nc.vector.scalar_tensor_tensor(
                out=ut[:], in0=ps2[:], scalar=it[:], in1=av[:],
                op0=mybir.AluOpType.mult, op1=mybir.AluOpType.add,
            )
            ea = sb.tile([BS, O2], f32, tag="ea")
            nc.scalar.activation(out=ea[:], in_=ut[:],
                                 func=mybir.ActivationFunctionType.Exp)
            den = sb.tile([BS, U], f32, tag="den")
            nc.vector.tensor_reduce(
                out=den[:],
                in_=ea[:].rearrange("p (u n) -> p u n", n=N),
                axis=mybir.AxisListType.X,
                op=mybir.AluOpType.add,
            )
            rec = sb.tile([BS, U], f32, tag="rec")
            nc.vector.reciprocal(out=rec[:], in_=den[:])
            idt = sb.tile([BS, BS], f32, tag="idt")
            nc.sync.dma_start(out=idt[:], in_=ident)
            psE = pp.tile([O2, BS], f32, tag="psE")
            nc.tensor.transpose(psE[:], ea[:], idt[:])
            etE = sb.tile([O2, BS], f32, tag="etE")
            nc.vector.tensor_copy(out=etE[:], in_=psE[:])
            psR = pp.tile([U, BS], f32, tag="psR")
            nc.tensor.transpose(psR[:], rec[:], idt[:])
            etR = sb.tile([U, BS], f32, tag="etR")
            nc.vector.tensor_copy(out=etR[:], in_=psR[:])
            mk = sb.tile([O2, U], f32, tag="mk")
            nc.sync.dma_start(out=mk[:], in_=mask)
            for b in range(BS):
                inb = ib.tile([O2, F], f32, tag="inb")
                nc.sync.dma_start(out=inb[:], in_=minp[b])
                pb = sb.tile([O2, U], f32, tag=f"pb{b}")
                # block-diagonal unnormalized prob: mask[un,u] * exp[un]
                nc.vector.tensor_scalar_mul(pb[:], mk[:], etE[:, b:b + 1])
                ob = sb.tile([U, F], f32, tag=f"ob{b}")
                for f0 in range(0, F, FCH):
                    w = min(FCH, F - f0)
                    psf = pp.tile([U, FCH], f32, tag="psf", bufs=4)
                    nc.tensor.matmul(psf[:, :w], lhsT=pb[:],
                                     rhs=inb[:, f0:f0 + w],
                                     start=True, stop=True)
                    # normalize by 1/den[u] during PSUM->SBUF copy
                    nc.vector.tensor_scalar_mul(
                        ob[:, f0:f0 + w], psf[:, :w], etR[:, b:b + 1])
                nc.sync.dma_start(out=dout[b], in_=ob[:])
    nc.compile()
    return nc


def _get_compiled():
    if "a" not in _cache:
        _cache["a"] = _build_phase_a()
    if "b" not in _cache:
        _cache["b"] = _build_phase_b()
    return _cache["a"], _cache["b"]


def _run(nc, in_maps, label):
    kw = {}
    if PROFILE:
        _register_profile_hook()
        kw = dict(trace=True, tmpdir=tempfile.mkdtemp(prefix=f"bass_{label}_"))
    res = run_bass_kernel_spmd(nc, in_maps, core_ids=list(range(NCORES)), **kw)
    if PROFILE:
        LAST_EXEC_NS[label] = res.exec_time_ns
    return res.results


def kernel(input, temp, W1, b1, gamma, beta, rmean, rvar, W2, b2, gumbel):
    input = np.ascontiguousarray(np.asarray(input, dtype=np.float32))
    temp = np.float32(np.asarray(temp))
    W1 = np.asarray(W1, dtype=np.float32)
    b1 = np.asarray(b1, dtype=np.float32)
    gamma = np.asarray(gamma, dtype=np.float32)
    beta = np.asarray(beta, dtype=np.float32)
    rmean = np.asarray(rmean, dtype=np.float32)
    rvar = np.asarray(rvar, dtype=np.float32)
    W2 = np.asarray(W2, dtype=np.float32)
    b2 = np.asarray(b2, dtype=np.float32)
    gumbel = np.asarray(gumbel, dtype=np.float32)

    nca, ncb = _get_compiled()

    # ---- host prep, phase A ----
    x2 = input.reshape(B, K1)
    xTp = np.zeros((K1P, B), np.float16)
    xTp[:K1] = x2.T
    xsb = np.ascontiguousarray(
        xTp.reshape(NKT, KT, B).transpose(1, 0, 2)).reshape(KT, NKT * B)

    s = (gamma.astype(np.float64) / np.sqrt(rvar.astype(np.float64) + BN_EPS))
    tv = s * (b1.astype(np.float64) - rmean.astype(np.float64)) \
        + beta.astype(np.float64)
    s = s.astype(np.float32)
    tv = tv.astype(np.float32)

    W1_2d = W1.reshape(O1, K1)
    NFC = K1 // (G * KT)          # 148 chunks fully covered by real rows
    NFT = K1 // KT                # 2373 full 128-row k-tiles
    in_maps_a = []
    for i in range(NCORES):
        o0 = i * OS
        o1 = min(o0 + OS, O1)
        ow = o1 - o0
        w1t_i = np.zeros((NCH, KT, G, OS), np.float16)
        srcT = W1_2d[o0:o1].T     # [K1, ow] strided view
        src4 = srcT[:NFC * G * KT].reshape(NFC, G, KT, ow)
        for g in range(G):
            w1t_i[:NFC, :, g, :ow] = src4[:, g]
        for t in range(NFC * G, NFT + 1):
            k0 = t * KT
            kw = min(KT, K1 - k0)
            if kw > 0:
                w1t_i[NFC, :kw, t - NFC * G, :ow] = srcT[k0:k0 + kw]
        w1t_i = w1t_i.reshape(NCH, KT, G * OS)
        sp = np.zeros((OS,), np.float32)
        sp[:ow] = s[o0:o1]
        tp = np.zeros((OS,), np.float32)
        tp[:ow] = tv[o0:o1]
        in_maps_a.append({
            "w1t": w1t_i,
            "xsb": xsb,
            "ssb": np.ascontiguousarray(np.repeat(sp[None], B, 0)),
            "tsb": np.ascontiguousarray(np.repeat(tp[None], B, 0)),
        })

    res_a = _run(nca, in_maps_a, "phase_a")
    h_full = np.concatenate([r["h"] for r in res_a], axis=1)[:, :O1]

    # ---- host prep, phase B ----
    hT = np.zeros((K2P, B), np.float32)
    hT[:O1] = h_full.T
    hT_r = np.ascontiguousarray(hT.reshape(NKT2, KT, B).transpose(1, 0, 2))
    W2_2d = W2.reshape(O2, O1)
    w2T = np.zeros((K2P, O2), np.float32)
    w2T[:O1] = W2_2d.T
    w2sb = np.ascontiguousarray(
        w2T.reshape(NKT2, KT, O2).transpose(1, 0, 2)).reshape(KT, NKT2 * O2)
    inv_t = np.float32(1.0) / temp
    gum2 = gumbel.reshape(B, O2)
    addv_all = (b2[None, :] + gum2) * inv_t
    maskm = np.zeros((O2, U), np.float32)
    maskm[np.arange(O2), np.arange(O2) // N] = 1.0
    identm = np.eye(BS, dtype=np.float32)
    itempm = np.full((BS, 1), inv_t, np.float32)
    inp81 = input.reshape(B, O2, F)

    in_maps_b = []
    for i in range(NCORES):
        b0 = i * BS
        hsb_i = np.ascontiguousarray(
            hT_r[:, :, b0:b0 + BS]).reshape(KT, NKT2 * BS)
        in_maps_b.append({
            "hsb": hsb_i,
            "w2sb": w2sb,
            "addv": np.ascontiguousarray(addv_all[b0:b0 + BS]),
            "itemp": itempm,
            "minp": np.ascontiguousarray(inp81[b0:b0 + BS]),
            "mask": maskm,
            "ident": identm,
        })

    res_b = _run(ncb, in_maps_b, "phase_b")
    out = np.concatenate([r["dot"] for r in res_b], axis=0)
    return out.reshape(B, U, A, F)


# revision 25
# speedup vs baseline: 1.2782x; 1.0553x over previous
"""Trainium2 Bass kernel for nn_AttentionEncoder (8-core SPMD).

Structure:
  Phase A (tensor-parallel over conv1 output channels):
    h[b, o] = sum_k x[b, k] * W1[o, k]  -- the 3.28 GB W1 stream dominates
    (memory-bound).  W1 is sharded into 8 x [338, 303750] output-channel
    slices; each core streams its slice (cast to fp16 on host, halving
    HBM traffic) through the PE as the moving operand while the (tiny)
    transposed x is the stationary operand.  BatchNorm (+conv1 bias) is
    folded into a per-channel scale/bias applied in the epilogue.
  Phase B (data-parallel over batch, 2 per core):
    logits = h @ W2.T + b2 ; gumbel-softmax over N=9 ; attention pooling
    dot[b,u,f] = sum_n prob[b,u,n] * input[b,u,n,f] done as a block-
    diagonal [81x9] matmul against input[b] viewed as [81, 3750].
"""

import os
import sys
import tempfile

import numpy as np

for _p in ("/opt/trn_rl_repo", "/root/.axon_site/_ro/trn_rl_repo"):
    if os.path.isdir(_p) and _p not in sys.path:
        sys.path.append(_p)

import concourse.tile as tile
from concourse import bacc, mybir
from concourse.bass_utils import run_bass_kernel_spmd

# ---- problem constants (hardcoded; kernel.py must be self-contained) ----
B, U, A, N, F, L = 16, 9, 1, 9, 3750, 300
K1 = U * N * F            # 303750  conv1 contraction
O1 = U * L                # 2700    conv1 output channels
O2 = U * A * N            # 81      conv2 output channels
BN_EPS = 1e-5
NCORES = 8
OS = 338                  # per-core conv1 output-channel shard (8*338=2704)
KT = 128                  # PE contraction tile
G = 32                    # k-tiles per W1 DMA chunk (2.77 MB per chunk)
NKT = (K1 + KT - 1) // KT  # 2374 real k-tiles (last one row-padded)
NCH = (NKT + G - 1) // G  # W1 chunks (last one holds a partial tile count)
K1P = NKT * KT            # 303872
BS = B // NCORES          # 2 batches per core in phase B
NKT2 = (O1 + KT - 1) // KT  # 22
K2P = NKT2 * KT           # 2816
FCH = 512                 # pooling free-dim chunk (one PSUM bank of fp32)

PROFILE = os.environ.get("BASS_KERNEL_PROFILE", "0") == "1"
LAST_EXEC_NS = {}

_cache = {}


def _register_profile_hook():
    """boot() skips NTFF hook registration when antenv.axon_hooks is absent;
    recreate the module and register the ctypes-based hook ourselves."""
    import types

    if "antenv.axon_hooks" in sys.modules:
        return
    mod = types.ModuleType("antenv.axon_hooks")
    _hook = [None]
    mod.set_axon_ntff_profile_hook = lambda h: _hook.__setitem__(0, h)
    mod.get_axon_ntff_profile_hook = lambda: _hook[0]
    sys.modules["antenv.axon_hooks"] = mod
    import antenv

    antenv.axon_hooks = mod
    try:
        from trn_agent_boot.trn_boot import _ntff_profile_via_ctypes

        mod.set_axon_ntff_profile_hook(
            _ntff_profile_via_ctypes("/opt/axon/libaxon_pjrt.so")
        )
    except Exception:
        pass
    import concourse.bass_utils as bu

    bu.upload_artifacts = lambda tmpdir: "local://" + tmpdir


def _build_phase_a():
    nc = bacc.Bacc("TRN2", target_bir_lowering=False, debug=False,
                   num_devices=NCORES)
    f16, f32 = mybir.dt.float16, mybir.dt.float32
    w1t = nc.dram_tensor("w1t", [NCH, KT, G * OS], f16, kind="ExternalInput").ap()
    xsb = nc.dram_tensor("xsb", [KT, NKT * B], f16, kind="ExternalInput").ap()
    ssb = nc.dram_tensor("ssb", [B, OS], f32, kind="ExternalInput").ap()
    tsb = nc.dram_tensor("tsb", [B, OS], f32, kind="ExternalInput").ap()
    hout = nc.dram_tensor("h", [B, OS], f32, kind="ExternalOutput").ap()

    with tile.TileContext(nc) as tc:
        with tc.tile_pool(name="xp", bufs=1) as xp, \
             tc.tile_pool(name="wp", bufs=4) as wp, \
             tc.tile_pool(name="pp", bufs=1, space="PSUM") as pp, \
             tc.tile_pool(name="ep", bufs=1) as ep:
            xt = xp.tile([KT, NKT * B], f16)
            nc.sync.dma_start(out=xt[:], in_=xsb)
            psum = pp.tile([B, OS], f32)
            for c in range(NCH):
                gg = min(G, NKT - c * G)
                wt = wp.tile([KT, G * OS], f16, tag="wt")
                if gg == G:
                    nc.sync.dma_start(out=wt[:], in_=w1t[c])
                else:
                    nc.sync.dma_start(out=wt[:, :gg * OS],
                                      in_=w1t[c][:, :gg * OS])
                for g in range(gg):
                    t = c * G + g
                    nc.tensor.matmul(
                        psum[:],
                        lhsT=xt[:, t * B:(t + 1) * B],
                        rhs=wt[:, g * OS:(g + 1) * OS],
                        start=(t == 0),
                        stop=(t == NKT - 1),
                    )
            st = ep.tile([B, OS], f32, tag="st")
            nc.sync.dma_start(out=st[:], in_=ssb)
            tt = ep.tile([B, OS], f32, tag="tt")
            nc.sync.dma_start(out=tt[:], in_=tsb)
            ho = ep.tile([B, OS], f32, tag="ho")
            nc.vector.tensor_mul(out=ho[:], in0=psum[:], in1=st[:])
            nc.vector.tensor_add(out=ho[:], in0=ho[:], in1=tt[:])
            nc.sync.dma_start(out=hout, in_=ho[:])
    nc.compile()
    return nc


def _build_phase_b():
    nc = bacc.Bacc("TRN2", target_bir_lowering=False, debug=False,
                   num_devices=NCORES)
    f16, f32 = mybir.dt.float16, mybir.dt.float32
    hsb = nc.dram_tensor("hsb", [KT, NKT2 * BS], f32, kind="ExternalInput").ap()
    w2sb = nc.dram_tensor("w2sb", [KT, NKT2 * O2], f32, kind="ExternalInput").ap()
    addv = nc.dram_tensor("addv", [BS, O2], f32, kind="ExternalInput").ap()
    itemp = nc.dram_tensor("itemp", [BS, 1], f32, kind="ExternalInput").ap()
    minp = nc.dram_tensor("minp", [BS, O2, F], f16, kind="ExternalInput").ap()
    mask = nc.dram_tensor("mask", [O2, U], f16, kind="ExternalInput").ap()
    ident = nc.dram_tensor("ident", [BS, BS], f32, kind="ExternalInput").ap()
    dout = nc.dram_tensor("dot", [BS, U, F], f32, kind="ExternalOutput").ap()



    with tile.TileContext(nc) as tc:
        with tc.tile_pool(name="sb", bufs=1) as sb, \
             tc.tile_pool(name="inb", bufs=2) as ib:
            hs = sb.tile([KT, NKT2 * BS], f32, tag="hs")
            nc.sync.dma_start(out=hs[:], in_=hsb)
            w2 = sb.tile([KT, NKT2 * O2], f32, tag="w2")
            nc.sync.dma_start(out=w2[:], in_=w2sb)
            av = sb.tile([BS, O2], f32, tag="av")
            nc.sync.dma_start(out=av[:], in_=addv)
            it = sb.tile([BS, 1], f32, tag="it")
            nc.sync.dma_start(out=it[:], in_=itemp)
            idt = sb.tile([BS, BS], f32, tag="idt")
            nc.sync.dma_start(out=idt[:], in_=ident)
            mk = sb.tile([O2, U], f16, tag="mk")
            nc.sync.dma_start(out=mk[:], in_=mask)
            inbs = []
            for b in range(BS):
                inb = ib.tile([O2, F], f16, tag="inb")
                nc.sync.dma_start(out=inb[:], in_=minp[b])
                inbs.append(inb)
            etP = sb.tile([O2, BS], f32, tag="etP")
            with tc.tile_pool(name="pp1", bufs=1, space="PSUM") as pp1:
                ps2 = pp1.tile([BS, O2], f32, tag="ps2")
                for t in range(NKT2):
                    nc.tensor.matmul(
                        ps2[:],
                        lhsT=hs[:, t * BS:(t + 1) * BS],
                        rhs=w2[:, t * O2:(t + 1) * O2],
                        start=(t == 0),
                        stop=(t == NKT2 - 1),
                    )
                ut = sb.tile([BS, O2], f32, tag="ut")
                # u = (logits * (1/temp)) + (b2 + gumbel)/temp
                nc.vector.scalar_tensor_tensor(
                    out=ut[:], in0=ps2[:], scalar=it[:], in1=av[:],
                    op0=mybir.AluOpType.mult, op1=mybir.AluOpType.add,
                )
                ea = sb.tile([BS, O2], f32, tag="ea")
                nc.scalar.activation(out=ea[:], in_=ut[:],
                                     func=mybir.ActivationFunctionType.Exp)
                den = sb.tile([BS, U], f32, tag="den")
                ea3 = ea[:].rearrange("p (u n) -> p u n", n=N)
                nc.vector.tensor_reduce(
                    out=den[:], in_=ea3,
                    axis=mybir.AxisListType.X, op=mybir.AluOpType.add,
                )
                rec = sb.tile([BS, U], f32, tag="rec")
                nc.vector.reciprocal(out=rec[:], in_=den[:])
                prob = sb.tile([BS, O2], f32, tag="prob")
                prob3 = prob[:].rearrange("p (u n) -> p u n", n=N)
                try:
                    rb = rec[:].unsqueeze(2).broadcast_to((BS, U, N))
                    nc.vector.tensor_mul(out=prob3, in0=ea3, in1=rb)
                except Exception:
                    for n_ in range(N):
                        nc.vector.tensor_mul(out=prob3[:, :, n_],
                                             in0=ea3[:, :, n_], in1=rec[:])
                psE = pp1.tile([O2, BS], f32, tag="psE")
                nc.tensor.transpose(psE[:], prob[:], idt[:])
                nc.vector.tensor_copy(out=etP[:], in_=psE[:])
            with tc.tile_pool(name="pp2", bufs=1, space="PSUM") as pp2:
                HF = 1920  # vector/scalar epilogue split point (psum bank aligned)
                for b in range(BS):
                    pb = sb.tile([O2, U], f16, tag=f"pb{b}")
                    nc.vector.tensor_scalar_mul(pb[:], mk[:], etP[:, b:b + 1])
                    psf = pp2.tile([U, F], f32, tag="psf")
                    for f0 in range(0, F, FCH):
                        w = min(FCH, F - f0)
                        nc.tensor.matmul(psf[:, f0:f0 + w], lhsT=pb[:],
                                         rhs=inbs[b][:, f0:f0 + w],
                                         start=True, stop=True)
                    ob = sb.tile([U, F], f32, tag=f"ob{b}")
                    nc.vector.tensor_copy(out=ob[:, :HF], in_=psf[:, :HF])
                    nc.scalar.copy(out=ob[:, HF:], in_=psf[:, HF:])
                    nc.sync.dma_start(out=dout[b], in_=ob[:])
    nc.compile()
    return nc


def _get_compiled():
    if "a" not in _cache:
        _cache["a"] = _build_phase_a()
    if "b" not in _cache:
        _cache["b"] = _build_phase_b()
    return _cache["a"], _cache["b"]


def _run(nc, in_maps, label):
    kw = {}
    if PROFILE:
        _register_profile_hook()
        kw = dict(trace=True, tmpdir=tempfile.mkdtemp(prefix=f"bass_{label}_"))
    res = run_bass_kernel_spmd(nc, in_maps, core_ids=list(range(NCORES)), **kw)
    if PROFILE:
        LAST_EXEC_NS[label] = res.exec_time_ns
    return res.results


def kernel(input, temp, W1, b1, gamma, beta, rmean, rvar, W2, b2, gumbel):
    input = np.ascontiguousarray(np.asarray(input, dtype=np.float32))
    temp = np.float32(np.asarray(temp))
    W1 = np.asarray(W1, dtype=np.float32)
    b1 = np.asarray(b1, dtype=np.float32)
    gamma = np.asarray(gamma, dtype=np.float32)
    beta = np.asarray(beta, dtype=np.float32)
    rmean = np.asarray(rmean, dtype=np.float32)
    rvar = np.asarray(rvar, dtype=np.float32)
    W2 = np.asarray(W2, dtype=np.float32)
    b2 = np.asarray(b2, dtype=np.float32)
    gumbel = np.asarray(gumbel, dtype=np.float32)

    nca, ncb = _get_compiled()

    # ---- host prep, phase A ----
    x2 = input.reshape(B, K1)
    xTp = np.zeros((K1P, B), np.float16)
    xTp[:K1] = x2.T
    xsb = np.ascontiguousarray(
        xTp.reshape(NKT, KT, B).transpose(1, 0, 2)).reshape(KT, NKT * B)

    s = (gamma.astype(np.float64) / np.sqrt(rvar.astype(np.float64) + BN_EPS))
    tv = s * (b1.astype(np.float64) - rmean.astype(np.float64)) \
        + beta.astype(np.float64)
    s = s.astype(np.float32)
    tv = tv.astype(np.float32)

    W1_2d = W1.reshape(O1, K1)
    NFC = K1 // (G * KT)          # 148 chunks fully covered by real rows
    NFT = K1 // KT                # 2373 full 128-row k-tiles
    in_maps_a = []
    for i in range(NCORES):
        o0 = i * OS
        o1 = min(o0 + OS, O1)
        ow = o1 - o0
        w1t_i = np.zeros((NCH, KT, G, OS), np.float16)
        srcT = W1_2d[o0:o1].T     # [K1, ow] strided view
        src4 = srcT[:NFC * G * KT].reshape(NFC, G, KT, ow)
        for g in range(G):
            w1t_i[:NFC, :, g, :ow] = src4[:, g]
        for t in range(NFC * G, NFT + 1):
            k0 = t * KT
            kw = min(KT, K1 - k0)
            if kw > 0:
                w1t_i[NFC, :kw, t - NFC * G, :ow] = srcT[k0:k0 + kw]
        w1t_i = w1t_i.reshape(NCH, KT, G * OS)
        sp = np.zeros((OS,), np.float32)
        sp[:ow] = s[o0:o1]
        tp = np.zeros((OS,), np.float32)
        tp[:ow] = tv[o0:o1]
        in_maps_a.append({
            "w1t": w1t_i,
            "xsb": xsb,
            "ssb": np.ascontiguousarray(np.repeat(sp[None], B, 0)),
            "tsb": np.ascontiguousarray(np.repeat(tp[None], B, 0)),
        })

    res_a = _run(nca, in_maps_a, "phase_a")
    h_full = np.concatenate([r["h"] for r in res_a], axis=1)[:, :O1]

    # ---- host prep, phase B ----
    hT = np.zeros((K2P, B), np.float32)
    hT[:O1] = h_full.T
    hT_r = np.ascontiguousarray(hT.reshape(NKT2, KT, B).transpose(1, 0, 2))
    W2_2d = W2.reshape(O2, O1)
    w2T = np.zeros((K2P, O2), np.float32)
    w2T[:O1] = W2_2d.T
    w2sb = np.ascontiguousarray(
        w2T.reshape(NKT2, KT, O2).transpose(1, 0, 2)).reshape(KT, NKT2 * O2)
    inv_t = np.float32(1.0) / temp
    gum2 = gumbel.reshape(B, O2)
    addv_all = (b2[None, :] + gum2) * inv_t
    maskm = np.zeros((O2, U), np.float16)
    maskm[np.arange(O2), np.arange(O2) // N] = 1.0
    identm = np.eye(BS, dtype=np.float32)
    itempm = np.full((BS, 1), inv_t, np.float32)
    inp81 = input.reshape(B, O2, F).astype(np.float16)

    in_maps_b = []
    for i in range(NCORES):
        b0 = i * BS
        hsb_i = np.ascontiguousarray(
            hT_r[:, :, b0:b0 + BS]).reshape(KT, NKT2 * BS)
        in_maps_b.append({
            "hsb": hsb_i,
            "w2sb": w2sb,
            "addv": np.ascontiguousarray(addv_all[b0:b0 + BS]),
            "itemp": itempm,
            "minp": np.ascontiguousarray(inp81[b0:b0 + BS]),
            "mask": maskm,
            "ident": identm,
        })

    res_b = _run(ncb, in_maps_b, "phase_b")
    out = np.concatenate([r["dot"] for r in res_b], axis=0)
    return out.reshape(B, U, A, F)
